# revision 4
# baseline (speedup 1.0000x reference)
"""Permutation scatter: out[perm[i]] = inputs[i]  (B=131072, D=512, f32).

Since perm is a permutation, out[j] = inputs[inv_perm[j]] -- a pure row
gather.  Strategy: shard the OUTPUT rows across the 8 cores and replicate
the full input to every core.  Core k owns output rows [k*R, (k+1)*R) and
gathers its 16384 rows (2 KiB each) from its local replica with indirect
DMAs, then writes its output shard contiguously.  No collectives; per-core
HBM traffic is the minimum possible (32 MiB read + 32 MiB write).  The
host only computes the inverse permutation (index math); all payload
movement happens on-device.

HW contract for indirect DMA (probed): one index per partition, dest AP
[128, D].  So each gather chunk covers 128 output rows; 128 chunks/core.
The per-core index tensor is passed pre-transposed (idxT[p, c] =
inv_k[c*128 + p]) so a single contiguous [128, 128] SBUF tile holds one
chunk's indices per column.
"""

import numpy as np

B = 131072
D = 512
N_CORES = 8
R = B // N_CORES  # 16384 output rows per core
P = 128
NCH = R // P  # 128 chunks per core

DATA_BUFS = 12
USE_RAW = True
RAW_SLOTS = 24  # rotating [128, RAW_GROUP*D] tiles
RAW_GROUP = 1  # 128-row gathers per store (grouping measured no better)

USE_V2 = True
V2_HEAD = 2  # chunks in the first index load (gates gather 0); 1 would make
# a non-contiguous 4B/partition load, so 2 is the minimum contiguous slice
V2_STORE_SPLIT = True  # alternate stores between sync and scalar HWDGE queues
V2_SWDGE_QUEUES = 2  # alternate indirect gathers across SWDGE queues

_cached = None


def _indirect_dma_q(eng, out, in_, offset_col, queue):
    """indirect_dma_start with a selectable SWDGE queue (bass hardcodes
    qPoolDynamic); body mirrors bass.py's indirect_dma_start gather path."""
    import concourse.mybir as mybir

    assert isinstance(in_.offset, int) and in_.offset == 0
    out_ap = eng.lower_ap_dma(out, for_indirect_dma=True)
    in_ap = eng.lower_ap_dma(in_, for_indirect_dma=True)
    offset_ap = eng.lower_ap_dma(offset_col)[0]
    in_ap.append(offset_ap)

    coef = 1
    for i in range(1, len(in_.shape)):
        coef *= in_.shape[i]
    in_ap[0].dynamic_ap_info = mybir.DynamicAccessPatternInfo(
        c=0,
        actual_ap=out.ap,
        indirect_dim_max_index=in_.shape[0],
        offset_expr=[
            mybir.DynamicAccessPatternOffsetExpr(
                coef=coef,
                aff_expr=mybir.DynamicAccessPatternOffsetExprAffExpr(
                    kind="IndirectArgId", arg_id=1
                ),
            )
        ],
    )
    return eng.add_instruction(
        mybir.InstDMACopy(
            name=eng.bass.get_next_instruction_name(),
            queue=queue,
            mode="Copy",
            ins=in_ap,
            outs=out_ap,
            oob_is_err=True,
            cce_op=mybir.AluOpType.bypass,
        )
    )


def _build_nc(data_bufs=DATA_BUFS):
    import concourse.bacc as bacc
    import concourse.bass as bass
    import concourse.mybir as mybir
    import concourse.tile as tile

    nc = bacc.Bacc(
        "TRN2",
        target_bir_lowering=False,
        debug=False,
        num_devices=N_CORES,
    )

    x = nc.dram_tensor("x", [B, D], mybir.dt.float32, kind="ExternalInput")
    # idxT[p, c] = source row for output row c*128 + p (core-local)
    idxT = nc.dram_tensor("idxT", [P, NCH], mybir.dt.int32, kind="ExternalInput")
    y = nc.dram_tensor("y", [R, D], mybir.dt.float32, kind="ExternalOutput")

    y_r = y[:].rearrange("(c p) d -> c p d", p=P)

    with tile.TileContext(nc) as tc:
        with (
            tc.tile_pool(name="idxp", bufs=1) as ipool,
            tc.tile_pool(name="data", bufs=data_bufs) as dpool,
        ):
            it = ipool.tile([P, NCH], mybir.dt.int32)
            nc.sync.dma_start(out=it[:], in_=idxT[:])
            for c in range(NCH):
                dtile = dpool.tile([P, D], mybir.dt.float32)
                nc.gpsimd.indirect_dma_start(
                    out=dtile[:],
                    out_offset=None,
                    in_=x[:],
                    in_offset=bass.IndirectOffsetOnAxis(ap=it[:, c : c + 1], axis=0),
                )
                nc.sync.dma_start(out=y_r[c], in_=dtile[:])

    nc.compile()
    return nc


def _build_nc_raw(slots=RAW_SLOTS, group=RAW_GROUP):
    """Raw-Bass version (no TileContext): hand-rolled semaphores, minimal
    prologue/epilogue.  ``group`` 128-row gathers land in one [128, group*D]
    SBUF tile which is written back with a single large store (fewer SP
    instructions, bigger store descriptors).  ``slots`` tiles rotate."""
    from contextlib import ExitStack

    import concourse.bass as bass
    import concourse.mybir as mybir

    n_groups = NCH // group
    assert NCH % group == 0

    nc = bass.Bass(
        "TRN2",
        target_bir_lowering=False,
        debug=False,
        num_devices=N_CORES,
    )

    x = nc.dram_tensor("x", [B, D], mybir.dt.float32, kind="ExternalInput")
    idxT = nc.dram_tensor("idxT", [P, NCH], mybir.dt.int32, kind="ExternalInput")
    y = nc.dram_tensor("y", [R, D], mybir.dt.float32, kind="ExternalOutput")
    # Store target for group j: output rows [j*group*128, (j+1)*group*128),
    # with partition p holding the `group` CONSECUTIVE rows
    # [j*group*128 + p*group, j*group*128 + (p+1)*group) -- so each partition
    # writes one contiguous group*D*4-byte run (big store descriptors).
    # Gather g of the group fills tile columns [g*D, (g+1)*D), so its 128
    # indices must be inv_k[j*group*128 + p*group + g] (see _make_in_maps).
    y_r = y[:].rearrange("(j p g) d -> j p (g d)", p=P, g=group)

    with ExitStack() as ctx:
        it = ctx.enter_context(nc.sbuf_tensor("it", [P, NCH], mybir.dt.int32))
        dts = [
            ctx.enter_context(
                nc.sbuf_tensor(f"dt{i}", [P, group * D], mybir.dt.float32)
            )
            for i in range(slots)
        ]
        # Per-slot semaphores with exact thresholds (a single cumulative sem
        # is racy: completions from the 16 SDMA engines interleave across
        # successive DMAs).  A slot's store waits for all `group` gathers of
        # its round (full sum = race-free); the next round's gathers wait for
        # that store.
        isem = nc.alloc_semaphore("isem")
        isem2 = nc.alloc_semaphore("isem2")
        gsems = [nc.alloc_semaphore(f"gsem{i}") for i in range(slots)]
        ssems = [nc.alloc_semaphore(f"ssem{i}") for i in range(slots)]

        # Split the index load: a small head load unblocks the first gathers
        # ~1.5us earlier (the 64KB load's completion receipt gates gather 0).
        head_chunks = 8
        assert head_chunks % group == 0 and head_chunks < NCH

        def rounds(slot):  # number of groups handled by this slot
            return (n_groups - slot + slots - 1) // slots

        with nc.Block(no_gpsimd_drain=True) as block:

            @block.sync
            def _(sync):
                sync.dma_start(out=it[:, :head_chunks], in_=idxT[:, :head_chunks]).then_inc(isem, 16)
                sync.dma_start(out=it[:, head_chunks:], in_=idxT[:, head_chunks:]).then_inc(isem2, 16)
                for j in range(n_groups):
                    i, k = j % slots, j // slots
                    sync.wait_ge(gsems[i], (k + 1) * group * 16)
                    sync.dma_start(out=y_r[j], in_=dts[i][:]).then_inc(
                        ssems[i], 16
                    )
                for i in range(slots):
                    sync.wait_ge(ssems[i], rounds(i) * 16)
                sync.wait_ge(isem, 16)
                sync.wait_ge(isem2, 16)

            @block.gpsimd
            def _(g_):
                g_.wait_ge(isem, 16)
                for j in range(n_groups):
                    i, k = j % slots, j // slots
                    if j * group == head_chunks:
                        g_.wait_ge(isem2, 16)
                    if j >= slots:
                        g_.wait_ge(ssems[i], k * 16)
                    for g in range(group):
                        c = j * group + g
                        g_.indirect_dma_start(
                            out=dts[i][:, g * D : (g + 1) * D],
                            out_offset=None,
                            in_=x[:],
                            in_offset=bass.IndirectOffsetOnAxis(
                                ap=it[:, c : c + 1], axis=0
                            ),
                        ).then_inc(gsems[i], 16)

        # Block exit emitted per-engine drains + a sem-only barrier; all DMA
        # completions were explicitly waited above, so a plain range-clear
        # (no dge drain) suffices to make the NEFF re-executable.
        sem_nums = sorted(
            [isem.num, isem2.num]
            + [s.num for s in gsems]
            + [s.num for s in ssems]
        )
        assert sem_nums == list(range(sem_nums[0], sem_nums[-1] + 1))
        nc.gpsimd.sem_clear(range(sem_nums[0], sem_nums[-1] + 1))

    return nc


def _build_nc_raw2(slots=RAW_SLOTS):
    """v2: 1-chunk index head (earlier first gather), stores alternating
    between the sync and scalar HWDGE queues, and indirect gathers
    alternating across 2 SWDGE queues."""
    from contextlib import ExitStack

    import concourse.bass as bass
    import concourse.mybir as mybir

    nc = bass.Bass(
        "TRN2",
        target_bir_lowering=False,
        debug=False,
        num_devices=N_CORES,
        num_swdge_queues=V2_SWDGE_QUEUES,
    )

    x = nc.dram_tensor("x", [B, D], mybir.dt.float32, kind="ExternalInput")
    idxT = nc.dram_tensor("idxT", [P, NCH], mybir.dt.int32, kind="ExternalInput")
    y = nc.dram_tensor("y", [R, D], mybir.dt.float32, kind="ExternalOutput")
    y_r = y[:].rearrange("(c p) d -> c p d", p=P)

    with ExitStack() as ctx:
        it = ctx.enter_context(nc.sbuf_tensor("it", [P, NCH], mybir.dt.int32))
        dts = [
            ctx.enter_context(nc.sbuf_tensor(f"dt{i}", [P, D], mybir.dt.float32))
            for i in range(slots)
        ]
        isem = nc.alloc_semaphore("isem")
        isem2 = nc.alloc_semaphore("isem2")
        isem3 = nc.alloc_semaphore("isem3")
        gsems = [nc.alloc_semaphore(f"gsem{i}") for i in range(slots)]
        ssems = [nc.alloc_semaphore(f"ssem{i}") for i in range(slots)]

        head = V2_HEAD
        head2 = 16  # second index stage
        q_names = ["qPoolDynamic", "qPoolDynamic1", "qPoolDynamic2", "qPoolDynamic3"]

        def rounds(slot):
            return (NCH - slot + slots - 1) // slots

        def store_eng(j):  # which HWDGE engine stores chunk j
            return (j % 2) if V2_STORE_SPLIT else 0

        def emit_stores(eng, eng_id):
            for j in range(NCH):
                if store_eng(j) != eng_id:
                    continue
                i, k = j % slots, j // slots
                eng.wait_ge(gsems[i], (k + 1) * 16)
                eng.dma_start(out=y_r[j], in_=dts[i][:]).then_inc(ssems[i], 16)
            for i in range(slots):
                if any(store_eng(j) == eng_id for j in range(i, NCH, slots)):
                    eng.wait_ge(ssems[i], rounds(i) * 16)

        with nc.Block(no_gpsimd_drain=True) as block:

            @block.sync
            def _(sync):
                sync.dma_start(out=it[:, :head], in_=idxT[:, :head]).then_inc(
                    isem, 16
                )
                sync.dma_start(
                    out=it[:, head:head2], in_=idxT[:, head:head2]
                ).then_inc(isem2, 16)
                sync.dma_start(out=it[:, head2:], in_=idxT[:, head2:]).then_inc(
                    isem3, 16
                )
                emit_stores(sync, 0)
                sync.wait_ge(isem, 16)
                sync.wait_ge(isem2, 16)
                sync.wait_ge(isem3, 16)

            if V2_STORE_SPLIT:

                @block.scalar
                def _(scalar):
                    emit_stores(scalar, 1)

            @block.gpsimd
            def _(g_):
                g_.wait_ge(isem, 16)
                for j in range(NCH):
                    i, k = j % slots, j // slots
                    if j == head:
                        g_.wait_ge(isem2, 16)
                    if j == head2:
                        g_.wait_ge(isem3, 16)
                    if j >= slots:
                        g_.wait_ge(ssems[i], k * 16)
                    _indirect_dma_q(
                        g_,
                        dts[i][:],
                        x[:],
                        it[:, j : j + 1],
                        q_names[j % V2_SWDGE_QUEUES],
                    ).then_inc(gsems[i], 16)

        sem_nums = sorted(
            [isem.num, isem2.num, isem3.num]
            + [s.num for s in gsems]
            + [s.num for s in ssems]
        )
        assert sem_nums == list(range(sem_nums[0], sem_nums[-1] + 1))
        nc.gpsimd.sem_clear(range(sem_nums[0], sem_nums[-1] + 1))

    return nc


def _get_nc():
    global _cached
    if _cached is None:
        if USE_V2:
            _cached = _build_nc_raw2()
        else:
            _cached = _build_nc_raw() if USE_RAW else _build_nc()
    return _cached


def _make_in_maps(inputs, perm):
    x = np.ascontiguousarray(np.asarray(inputs, dtype=np.float32))
    p = np.asarray(perm).astype(np.int64)
    inv = np.empty(B, dtype=np.int32)
    inv[p] = np.arange(B, dtype=np.int32)
    maps = []
    for k in range(N_CORES):
        sl = inv[k * R : (k + 1) * R]
        if USE_RAW:
            # idxT[p, j*group + g] = inv_k[j*group*128 + p*group + g]
            n_groups = NCH // RAW_GROUP
            idxT = (
                sl.reshape(n_groups, P, RAW_GROUP)
                .transpose(1, 0, 2)
                .reshape(P, NCH)
            )
        else:
            # idxT[p, c] = inv_k[c*128 + p]
            idxT = sl.reshape(NCH, P).T
        maps.append({"x": x, "idxT": np.ascontiguousarray(idxT)})
    return maps


def kernel(**kw):
    from concourse.bass_utils import run_bass_kernel_spmd

    nc = _get_nc()
    in_maps = _make_in_maps(kw["inputs"], kw["perm"])
    res = run_bass_kernel_spmd(nc, in_maps, core_ids=list(range(N_CORES)))
    return np.concatenate([res.results[k]["y"] for k in range(N_CORES)], axis=0)


def run_traced(inputs, perm, **trace_kw):
    """test.py helper: same as kernel() but returns (out, BassKernelResults)."""
    from concourse.bass_utils import run_bass_kernel_spmd

    nc = _get_nc()
    in_maps = _make_in_maps(inputs, perm)
    res = run_bass_kernel_spmd(
        nc, in_maps, core_ids=list(range(N_CORES)), trace=True, **trace_kw
    )
    out = np.concatenate([res.results[k]["y"] for k in range(N_CORES)], axis=0)
    return out, res



# revision 6
# speedup vs baseline: 1.0238x; 1.0238x over previous
"""Permutation scatter: out[perm[i]] = inputs[i]  (B=131072, D=512, f32).

Since perm is a permutation, out[j] = inputs[inv_perm[j]] -- a pure row
gather.  Strategy: shard the OUTPUT rows across the 8 cores and replicate
the full input to every core.  Core k owns output rows [k*R, (k+1)*R) and
gathers its 16384 rows (2 KiB each) from its local replica with indirect
DMAs, then writes its output shard contiguously.  No collectives; per-core
HBM traffic is the minimum possible (32 MiB read + 32 MiB write).  The
host only computes the inverse permutation (index math); all payload
movement happens on-device.

HW contract for indirect DMA (probed): one index per partition, dest AP
[128, D].  So each gather chunk covers 128 output rows; 128 chunks/core.
The per-core index tensor is passed pre-transposed (idxT[p, c] =
inv_k[c*128 + p]) so a single contiguous [128, 128] SBUF tile holds one
chunk's indices per column.
"""

import numpy as np

B = 131072
D = 512
N_CORES = 8
R = B // N_CORES  # 16384 output rows per core
P = 128
NCH = R // P  # 128 chunks per core

DATA_BUFS = 12
USE_RAW = True
RAW_SLOTS = 24  # rotating [128, RAW_GROUP*D] tiles
RAW_GROUP = 1  # 128-row gathers per store (grouping measured no better)

USE_V2 = True
V2_HEAD = 2  # chunks in the first index load (gates gather 0); 1 would make
# a non-contiguous 4B/partition load, so 2 is the minimum contiguous slice
V2_STORE_SPLIT = False  # measured +3.4us: 3-way queue RR starves gather drain
V2_SWDGE_QUEUES = 1  # measured no emission overlap (Pool engine serializes)
V2_SINGLE_PACKET = True  # pack each op's descs into one packet per engine

_cached = None


def _indirect_dma_q(eng, out, in_, offset_col, queue):
    """indirect_dma_start with a selectable SWDGE queue (bass hardcodes
    qPoolDynamic); body mirrors bass.py's indirect_dma_start gather path."""
    import concourse.mybir as mybir

    assert isinstance(in_.offset, int) and in_.offset == 0
    out_ap = eng.lower_ap_dma(out, for_indirect_dma=True)
    in_ap = eng.lower_ap_dma(in_, for_indirect_dma=True)
    offset_ap = eng.lower_ap_dma(offset_col)[0]
    in_ap.append(offset_ap)

    coef = 1
    for i in range(1, len(in_.shape)):
        coef *= in_.shape[i]
    in_ap[0].dynamic_ap_info = mybir.DynamicAccessPatternInfo(
        c=0,
        actual_ap=out.ap,
        indirect_dim_max_index=in_.shape[0],
        offset_expr=[
            mybir.DynamicAccessPatternOffsetExpr(
                coef=coef,
                aff_expr=mybir.DynamicAccessPatternOffsetExprAffExpr(
                    kind="IndirectArgId", arg_id=1
                ),
            )
        ],
    )
    return eng.add_instruction(
        mybir.InstDMACopy(
            name=eng.bass.get_next_instruction_name(),
            queue=queue,
            mode="Copy",
            ins=in_ap,
            outs=out_ap,
            oob_is_err=True,
            cce_op=mybir.AluOpType.bypass,
            single_packet=V2_SINGLE_PACKET,
        )
    )


def _build_nc(data_bufs=DATA_BUFS):
    import concourse.bacc as bacc
    import concourse.bass as bass
    import concourse.mybir as mybir
    import concourse.tile as tile

    nc = bacc.Bacc(
        "TRN2",
        target_bir_lowering=False,
        debug=False,
        num_devices=N_CORES,
    )

    x = nc.dram_tensor("x", [B, D], mybir.dt.float32, kind="ExternalInput")
    # idxT[p, c] = source row for output row c*128 + p (core-local)
    idxT = nc.dram_tensor("idxT", [P, NCH], mybir.dt.int32, kind="ExternalInput")
    y = nc.dram_tensor("y", [R, D], mybir.dt.float32, kind="ExternalOutput")

    y_r = y[:].rearrange("(c p) d -> c p d", p=P)

    with tile.TileContext(nc) as tc:
        with (
            tc.tile_pool(name="idxp", bufs=1) as ipool,
            tc.tile_pool(name="data", bufs=data_bufs) as dpool,
        ):
            it = ipool.tile([P, NCH], mybir.dt.int32)
            nc.sync.dma_start(out=it[:], in_=idxT[:])
            for c in range(NCH):
                dtile = dpool.tile([P, D], mybir.dt.float32)
                nc.gpsimd.indirect_dma_start(
                    out=dtile[:],
                    out_offset=None,
                    in_=x[:],
                    in_offset=bass.IndirectOffsetOnAxis(ap=it[:, c : c + 1], axis=0),
                )
                nc.sync.dma_start(out=y_r[c], in_=dtile[:])

    nc.compile()
    return nc


def _build_nc_raw(slots=RAW_SLOTS, group=RAW_GROUP):
    """Raw-Bass version (no TileContext): hand-rolled semaphores, minimal
    prologue/epilogue.  ``group`` 128-row gathers land in one [128, group*D]
    SBUF tile which is written back with a single large store (fewer SP
    instructions, bigger store descriptors).  ``slots`` tiles rotate."""
    from contextlib import ExitStack

    import concourse.bass as bass
    import concourse.mybir as mybir

    n_groups = NCH // group
    assert NCH % group == 0

    nc = bass.Bass(
        "TRN2",
        target_bir_lowering=False,
        debug=False,
        num_devices=N_CORES,
    )

    x = nc.dram_tensor("x", [B, D], mybir.dt.float32, kind="ExternalInput")
    idxT = nc.dram_tensor("idxT", [P, NCH], mybir.dt.int32, kind="ExternalInput")
    y = nc.dram_tensor("y", [R, D], mybir.dt.float32, kind="ExternalOutput")
    # Store target for group j: output rows [j*group*128, (j+1)*group*128),
    # with partition p holding the `group` CONSECUTIVE rows
    # [j*group*128 + p*group, j*group*128 + (p+1)*group) -- so each partition
    # writes one contiguous group*D*4-byte run (big store descriptors).
    # Gather g of the group fills tile columns [g*D, (g+1)*D), so its 128
    # indices must be inv_k[j*group*128 + p*group + g] (see _make_in_maps).
    y_r = y[:].rearrange("(j p g) d -> j p (g d)", p=P, g=group)

    with ExitStack() as ctx:
        it = ctx.enter_context(nc.sbuf_tensor("it", [P, NCH], mybir.dt.int32))
        dts = [
            ctx.enter_context(
                nc.sbuf_tensor(f"dt{i}", [P, group * D], mybir.dt.float32)
            )
            for i in range(slots)
        ]
        # Per-slot semaphores with exact thresholds (a single cumulative sem
        # is racy: completions from the 16 SDMA engines interleave across
        # successive DMAs).  A slot's store waits for all `group` gathers of
        # its round (full sum = race-free); the next round's gathers wait for
        # that store.
        isem = nc.alloc_semaphore("isem")
        isem2 = nc.alloc_semaphore("isem2")
        gsems = [nc.alloc_semaphore(f"gsem{i}") for i in range(slots)]
        ssems = [nc.alloc_semaphore(f"ssem{i}") for i in range(slots)]

        # Split the index load: a small head load unblocks the first gathers
        # ~1.5us earlier (the 64KB load's completion receipt gates gather 0).
        head_chunks = 8
        assert head_chunks % group == 0 and head_chunks < NCH

        def rounds(slot):  # number of groups handled by this slot
            return (n_groups - slot + slots - 1) // slots

        with nc.Block(no_gpsimd_drain=True) as block:

            @block.sync
            def _(sync):
                sync.dma_start(out=it[:, :head_chunks], in_=idxT[:, :head_chunks]).then_inc(isem, 16)
                sync.dma_start(out=it[:, head_chunks:], in_=idxT[:, head_chunks:]).then_inc(isem2, 16)
                for j in range(n_groups):
                    i, k = j % slots, j // slots
                    sync.wait_ge(gsems[i], (k + 1) * group * 16)
                    sync.dma_start(out=y_r[j], in_=dts[i][:]).then_inc(
                        ssems[i], 16
                    )
                for i in range(slots):
                    sync.wait_ge(ssems[i], rounds(i) * 16)
                sync.wait_ge(isem, 16)
                sync.wait_ge(isem2, 16)

            @block.gpsimd
            def _(g_):
                g_.wait_ge(isem, 16)
                for j in range(n_groups):
                    i, k = j % slots, j // slots
                    if j * group == head_chunks:
                        g_.wait_ge(isem2, 16)
                    if j >= slots:
                        g_.wait_ge(ssems[i], k * 16)
                    for g in range(group):
                        c = j * group + g
                        g_.indirect_dma_start(
                            out=dts[i][:, g * D : (g + 1) * D],
                            out_offset=None,
                            in_=x[:],
                            in_offset=bass.IndirectOffsetOnAxis(
                                ap=it[:, c : c + 1], axis=0
                            ),
                        ).then_inc(gsems[i], 16)

        # Block exit emitted per-engine drains + a sem-only barrier; all DMA
        # completions were explicitly waited above, so a plain range-clear
        # (no dge drain) suffices to make the NEFF re-executable.
        sem_nums = sorted(
            [isem.num, isem2.num]
            + [s.num for s in gsems]
            + [s.num for s in ssems]
        )
        assert sem_nums == list(range(sem_nums[0], sem_nums[-1] + 1))
        nc.gpsimd.sem_clear(range(sem_nums[0], sem_nums[-1] + 1))

    return nc


def _build_nc_raw2(slots=RAW_SLOTS):
    """v2: 1-chunk index head (earlier first gather), stores alternating
    between the sync and scalar HWDGE queues, and indirect gathers
    alternating across 2 SWDGE queues."""
    from contextlib import ExitStack

    import concourse.bass as bass
    import concourse.mybir as mybir

    nc = bass.Bass(
        "TRN2",
        target_bir_lowering=False,
        debug=False,
        num_devices=N_CORES,
        num_swdge_queues=V2_SWDGE_QUEUES,
    )

    x = nc.dram_tensor("x", [B, D], mybir.dt.float32, kind="ExternalInput")
    idxT = nc.dram_tensor("idxT", [P, NCH], mybir.dt.int32, kind="ExternalInput")
    y = nc.dram_tensor("y", [R, D], mybir.dt.float32, kind="ExternalOutput")
    y_r = y[:].rearrange("(c p) d -> c p d", p=P)

    with ExitStack() as ctx:
        it = ctx.enter_context(nc.sbuf_tensor("it", [P, NCH], mybir.dt.int32))
        dts = [
            ctx.enter_context(nc.sbuf_tensor(f"dt{i}", [P, D], mybir.dt.float32))
            for i in range(slots)
        ]
        isem = nc.alloc_semaphore("isem")
        isem2 = nc.alloc_semaphore("isem2")
        isem3 = nc.alloc_semaphore("isem3")
        gsems = [nc.alloc_semaphore(f"gsem{i}") for i in range(slots)]
        ssems = [nc.alloc_semaphore(f"ssem{i}") for i in range(slots)]

        head = V2_HEAD
        head2 = 16  # second index stage
        q_names = ["qPoolDynamic", "qPoolDynamic1", "qPoolDynamic2", "qPoolDynamic3"]

        def rounds(slot):
            return (NCH - slot + slots - 1) // slots

        def store_eng(j):  # which HWDGE engine stores chunk j
            return (j % 2) if V2_STORE_SPLIT else 0

        def emit_stores(eng, eng_id):
            for j in range(NCH):
                if store_eng(j) != eng_id:
                    continue
                i, k = j % slots, j // slots
                eng.wait_ge(gsems[i], (k + 1) * 16)
                eng.dma_start(out=y_r[j], in_=dts[i][:]).then_inc(ssems[i], 16)
            for i in range(slots):
                if any(store_eng(j) == eng_id for j in range(i, NCH, slots)):
                    eng.wait_ge(ssems[i], rounds(i) * 16)

        with nc.Block(no_gpsimd_drain=True) as block:

            @block.sync
            def _(sync):
                sync.dma_start(out=it[:, :head], in_=idxT[:, :head]).then_inc(
                    isem, 16
                )
                sync.dma_start(
                    out=it[:, head:head2], in_=idxT[:, head:head2]
                ).then_inc(isem2, 16)
                sync.dma_start(out=it[:, head2:], in_=idxT[:, head2:]).then_inc(
                    isem3, 16
                )
                emit_stores(sync, 0)
                sync.wait_ge(isem, 16)
                sync.wait_ge(isem2, 16)
                sync.wait_ge(isem3, 16)

            if V2_STORE_SPLIT:

                @block.scalar
                def _(scalar):
                    emit_stores(scalar, 1)

            @block.gpsimd
            def _(g_):
                g_.wait_ge(isem, 16)
                for j in range(NCH):
                    i, k = j % slots, j // slots
                    if j == head:
                        g_.wait_ge(isem2, 16)
                    if j == head2:
                        g_.wait_ge(isem3, 16)
                    if j >= slots:
                        g_.wait_ge(ssems[i], k * 16)
                    _indirect_dma_q(
                        g_,
                        dts[i][:],
                        x[:],
                        it[:, j : j + 1],
                        q_names[j % V2_SWDGE_QUEUES],
                    ).then_inc(gsems[i], 16)

        sem_nums = sorted(
            [isem.num, isem2.num, isem3.num]
            + [s.num for s in gsems]
            + [s.num for s in ssems]
        )
        assert sem_nums == list(range(sem_nums[0], sem_nums[-1] + 1))
        nc.gpsimd.sem_clear(range(sem_nums[0], sem_nums[-1] + 1))

    return nc


def _get_nc():
    global _cached
    if _cached is None:
        if USE_V2:
            _cached = _build_nc_raw2()
        else:
            _cached = _build_nc_raw() if USE_RAW else _build_nc()
    return _cached


def _make_in_maps(inputs, perm):
    x = np.ascontiguousarray(np.asarray(inputs, dtype=np.float32))
    p = np.asarray(perm).astype(np.int64)
    inv = np.empty(B, dtype=np.int32)
    inv[p] = np.arange(B, dtype=np.int32)
    maps = []
    for k in range(N_CORES):
        sl = inv[k * R : (k + 1) * R]
        if USE_RAW:
            # idxT[p, j*group + g] = inv_k[j*group*128 + p*group + g]
            n_groups = NCH // RAW_GROUP
            idxT = (
                sl.reshape(n_groups, P, RAW_GROUP)
                .transpose(1, 0, 2)
                .reshape(P, NCH)
            )
        else:
            # idxT[p, c] = inv_k[c*128 + p]
            idxT = sl.reshape(NCH, P).T
        maps.append({"x": x, "idxT": np.ascontiguousarray(idxT)})
    return maps


def kernel(**kw):
    from concourse.bass_utils import run_bass_kernel_spmd

    nc = _get_nc()
    in_maps = _make_in_maps(kw["inputs"], kw["perm"])
    res = run_bass_kernel_spmd(nc, in_maps, core_ids=list(range(N_CORES)))
    return np.concatenate([res.results[k]["y"] for k in range(N_CORES)], axis=0)


def run_traced(inputs, perm, **trace_kw):
    """test.py helper: same as kernel() but returns (out, BassKernelResults)."""
    from concourse.bass_utils import run_bass_kernel_spmd

    nc = _get_nc()
    in_maps = _make_in_maps(inputs, perm)
    res = run_bass_kernel_spmd(
        nc, in_maps, core_ids=list(range(N_CORES)), trace=True, **trace_kw
    )
    out = np.concatenate([res.results[k]["y"] for k in range(N_CORES)], axis=0)
    return out, res

